# revision 27
# baseline (speedup 1.0000x reference)
"""GQA attention (SEQ=2048, DIM=4096, 32 Q heads / 8 KV heads, head_dim=128),
tensor-parallel over heads across 8 NeuronCores.

Each core owns 4 Q heads + 1 KV head: wq/wk/wv split column-wise, wo split
row-wise; each core produces a partial (2048, 4096) output that the host sums
(the all-reduce of row-parallel wo).

~396us (vs the f32r baseline at 629us; PE roofline for this decomposition is
~389us: 166us QKV + 89us causal-trimmed attention + 111us out-projection +
~12us startup fill + ~9us teardown). Changes vs the baseline:
 - all matmul operands in bf16 (same PE rate as f32r, but FWL weight loads,
   half the DMA traffic and half the SBUF footprint; PSUM accum stays f32)
 - host pre-packs every DRAM tensor so each DMA's per-partition line is
   contiguous (>=4KB descriptors), x/out on the sync HWDGE ring, weights and
   tables on the scalar HWDGE ring
 - phase A runs K/V matmuls of each seq block before the Q matmuls and
   double-buffers the K/V PSUM banks, so the next block's PE work never waits
   on the PSUM eviction (RoPE) of the previous one; V^T->V transposes moved
   off the PE onto the DMA xbar (dma_start_transpose)
 - softmax denominator accumulated as broadcast rows via an all-ones lhsT
   (same PE cost, no separate 1->128 broadcast matmul) and inverted with
   reciprocal_approx_fast (~5x faster than reciprocal, which was a 3.3us
   critical-path stall per head)
 - phase C (out-projection) for query block qb is issued interleaved into the
   attention streams of qb+1, so the PE never idles on the normalization
   chain; PSUM evictions alternate between ACT and DVE
"""

import numpy as np
import ml_dtypes

import concourse.bacc as bacc
import concourse.tile as tile
from concourse import mybir
from concourse.bass_utils import run_bass_kernel_spmd

F32 = mybir.dt.float32
BF16 = mybir.dt.bfloat16
BF_NP = ml_dtypes.bfloat16

DIM = 4096
SEQ = 2048
HEAD_DIM = 128
N_CORES = 8
QH = 4              # q heads per core
QS = QH * HEAD_DIM  # 512: wq column slice per core
NKT = DIM // 128    # 32 contraction tiles
NSB = SEQ // 512    # 4 sequence blocks
NCH = 8             # x/weight super-chunks per seq block (4 k-tiles each)
SCALE = 1.0 / float(np.sqrt(HEAD_DIM))
NEG = -1e9
LAG = 5             # D/AV matmuls trail the score stream by LAG blocks


def build_nc():
    nc = bacc.Bacc(trn_type="TRN2")

    xP = nc.declare_dram_parameter("xP", [NSB * NCH * 128, 2048], BF16, isOutput=False)
    # weights packed so one DMA instruction covers multiple super-chunks
    # (the HWDGE ~600ns/instruction issue rate starves the startup otherwise)
    wqP = nc.declare_dram_parameter("wqP", [128, NCH * 2048], BF16, isOutput=False)
    wkvP = nc.declare_dram_parameter("wkvP", [128, NCH * 1024], BF16, isOutput=False)
    woP = nc.declare_dram_parameter("woP", [128, QH * DIM], BF16, isOutput=False)
    csP = nc.declare_dram_parameter("csP", [128, 2 * SEQ], F32, isOutput=False)
    stairP = nc.declare_dram_parameter("stairP", [128, 896], F32, isOutput=False)
    onesP = nc.declare_dram_parameter("onesP", [128, 128], BF16, isOutput=False)
    out = nc.declare_dram_parameter("out", [SEQ, DIM], BF16, isOutput=True)

    xr = xP.rearrange("(c p) s -> p c s", p=128)

    with tile.TileContext(nc) as tc:
        with (
            tc.tile_pool(name="persist", bufs=1) as persist,
            tc.tile_pool(name="resid", bufs=1) as resid,
        ):
            # stair/ones are only needed in phase B; they ride at the tail of
            # the scalar ring so the sync ring starts with the x stream
            stair_sb = persist.tile([128, 896], F32)
            ones_sb = persist.tile([128, 128], BF16)
            wo_sb = persist.tile([128, QH * DIM], BF16)

            # resident activations (all bf16)
            qT = resid.tile([128, QH, SEQ], BF16)      # Q^T per head (d, seq)
            kT = resid.tile([128, SEQ], BF16)          # K^T (d, seq)
            vN = resid.tile([128, SEQ // 128, 128], BF16)  # V natural (keys, d)

            # ---------------- Phase A: projections + RoPE ----------------
            with (
                tc.tile_pool(name="wpool", bufs=1) as wpool,
                tc.tile_pool(name="xpool", bufs=1) as xpool,
                tc.tile_pool(name="cspool", bufs=1) as cspool,
                tc.tile_pool(name="ropetmp", bufs=2) as ropetmp,
                tc.tile_pool(name="vtb", bufs=2) as vtb,
                tc.tile_pool(name="psQ", bufs=1, space="PSUM") as psQ,
                tc.tile_pool(name="psKV", bufs=2, space="PSUM") as psKV,
            ):
                cs_sb = cspool.tile([128, 2 * SEQ], F32)

                wq_sb = wpool.tile([128, NCH * 2048], BF16, name="wq")
                wkv_sb = wpool.tile([128, NCH * 1024], BF16, name="wkv")

                xts = [None] * NCH

                def rope(dst, src_ps, ss):
                    # rotate-half via two partition-offset PSUM->SBUF copies
                    # on ACT; multiplies/add on DVE (sin sign pre-folded)
                    vr = ropetmp.tile([128, 512], F32, tag="vr", name="vr")
                    nc.scalar.copy(vr[0:64, :], src_ps[64:128, :])
                    nc.scalar.copy(vr[64:128, :], src_ps[0:64, :])
                    t = ropetmp.tile([128, 512], F32, tag="t", name="t")
                    u = ropetmp.tile([128, 512], F32, tag="u", name="u")
                    nc.vector.tensor_mul(t, src_ps, cs_sb[:, ss])
                    nc.vector.tensor_mul(
                        u, vr, cs_sb[:, SEQ + ss.start:SEQ + ss.stop])
                    nc.vector.tensor_add(dst, t, u)

                for sb in range(NSB):
                    ss = slice(sb * 512, (sb + 1) * 512)
                    q_ps = [psQ.tile([128, 512], F32, tag=f"qps{h}", name=f"qps{h}")
                            for h in range(QH)]
                    k_ps = psKV.tile([128, 512], F32, tag="kps", name="kps")
                    v_ps = psKV.tile([128, 512], F32, tag="vps", name="vps")

                    # DMAs for this seq block: x chunks on the sync ring;
                    # (first block only) weights on the scalar ring in
                    # consumption order (kv pass g0-3 / q pass g0-3 / ...),
                    # then RoPE tables, then wo
                    for g in range(NCH):
                        xt = xpool.tile([128, 2048], BF16, tag=f"x{g}",
                                        name=f"x{g}")
                        if sb == 0 and g == 0:
                            # prime the pipe: the first matmuls only need the
                            # first k-tile slices; the first two wkv chunks
                            # ride this (otherwise idle) ring in between
                            for i4 in range(4):
                                sl = slice(i4 * 512, (i4 + 1) * 512)
                                nc.sync.dma_start(
                                    out=xt[:, sl], in_=xr[:, 0, sl])
                                if i4 < 2:
                                    kv1 = slice(i4 * 1024, (i4 + 1) * 1024)
                                    nc.sync.dma_start(out=wkv_sb[:, kv1],
                                                      in_=wkvP[:, kv1])
                        else:
                            nc.sync.dma_start(out=xt,
                                              in_=xr[:, sb * NCH + g, :])
                        xts[g] = xt
                    if sb == 0:
                        # weight slices in consumption order; first ones small
                        # so the very first matmuls start ASAP, later ones
                        # coalesced (HWDGE issues ~600ns/instruction)
                        def wkv_dma(glo, ghi):
                            sl = slice(glo * 1024, ghi * 1024)
                            nc.scalar.dma_start(out=wkv_sb[:, sl],
                                                in_=wkvP[:, sl])

                        def wq_dma(glo, ghi):
                            sl = slice(glo * 2048, ghi * 2048)
                            nc.scalar.dma_start(out=wq_sb[:, sl],
                                                in_=wqP[:, sl])

                        wq_dma(0, 1)
                        wkv_dma(2, 4)
                        wq_dma(1, 2)
                        wq_dma(2, 4)
                        wkv_dma(4, 6)
                        wkv_dma(6, 8)
                        wq_dma(4, 6)
                        wq_dma(6, 8)
                        nc.scalar.dma_start(out=cs_sb, in_=csP[:, :])
                        nc.scalar.dma_start(out=stair_sb, in_=stairP[:, :])
                        nc.scalar.dma_start(out=ones_sb, in_=onesP[:, :])
                        for c in range(2):
                            cs = slice(c * QH * DIM // 2, (c + 1) * QH * DIM // 2)
                            nc.scalar.dma_start(out=wo_sb[:, cs], in_=woP[:, cs])

                    def kv_pass(gs):
                        for g in gs:
                            for i in range(4):
                                kt_i = g * 4 + i
                                st = (kt_i == 0)
                                sp = (kt_i == NKT - 1)
                                xsl = xts[g][:, i * 512:(i + 1) * 512]
                                w0 = g * 1024 + i * 128
                                nc.tensor.matmul(
                                    k_ps, wkv_sb[:, w0:w0 + 128],
                                    xsl, start=st, stop=sp,
                                )
                                nc.tensor.matmul(
                                    v_ps, wkv_sb[:, w0 + 512:w0 + 640],
                                    xsl, start=st, stop=sp,
                                )

                    def q_pass(gs):
                        for g in gs:
                            for i in range(4):
                                kt_i = g * 4 + i
                                st = (kt_i == 0)
                                sp = (kt_i == NKT - 1)
                                xsl = xts[g][:, i * 512:(i + 1) * 512]
                                for h in range(QH):
                                    w0 = g * 2048 + i * 512 + h * 128
                                    nc.tensor.matmul(
                                        q_ps[h], wq_sb[:, w0:w0 + 128],
                                        xsl, start=st, stop=sp,
                                    )

                    # K/V of the next block never waits on Q evictions, and
                    # Q evictions overlap the second-half Q matmuls
                    kv_pass(range(0, 4))
                    q_pass(range(0, 4))
                    kv_pass(range(4, 8))

                    # K/V eviction (K rope + V transpose via DMA xbar),
                    # overlapped with the second-half Q matmuls below
                    rope(kT[:, ss], k_ps, ss)
                    vt_sb = vtb.tile([128, 512], BF16, tag="vt", name="vt")
                    nc.scalar.copy(vt_sb, v_ps)
                    for j in range(4):
                        nc.sync.dma_start_transpose(
                            out=vN[:, sb * 4 + j, :],
                            in_=vt_sb[:, j * 128:(j + 1) * 128],
                        )

                    # second-half Q pass head-major: head h's RoPE (which
                    # frees its PSUM bank and finishes its qT slice) runs
                    # under head h+1's matmul stream, so neither the next
                    # seq block nor phase B ever waits on an eviction tail
                    for h in range(QH):
                        for g in range(4, 8):
                            for i in range(4):
                                kt_i = g * 4 + i
                                w0 = g * 2048 + i * 512 + h * 128
                                nc.tensor.matmul(
                                    q_ps[h], wq_sb[:, w0:w0 + 128],
                                    xts[g][:, i * 512:(i + 1) * 512],
                                    start=False, stop=(kt_i == NKT - 1),
                                )
                        rope(qT[:, h, ss], q_ps[h], ss)

            # ---------------- Phase B/C: attention + out projection ----------------
            with (
                tc.tile_pool(name="expp", bufs=10) as expp,
                tc.tile_pool(name="otp", bufs=2) as otp,
                tc.tile_pool(name="rdp", bufs=2) as rdp,
                tc.tile_pool(name="outev", bufs=3) as outev,
                # pool-open order controls bank placement: psD/psOT/psC take
                # the q-accumulator banks (whose RoPE evictions drain last but
                # are only needed ~LAG blocks into phase B), psS gets the k/v
                # banks which phase A freed long before its end
                tc.tile_pool(name="psD", bufs=1, space="PSUM") as psD,
                tc.tile_pool(name="psOT", bufs=1, space="PSUM") as psOT,
                tc.tile_pool(name="psC", bufs=2, space="PSUM") as psC,
                tc.tile_pool(name="psS", bufs=4, space="PSUM") as psS,
            ):
                ot_hist = [[None] * QH for _ in range(NSB)]

                def c_chunk(qbc, qc, fine=False):
                    # out rows [qbc*512 + qc*128, +128) x all 4096 cols;
                    # fine=True drains quarter-DMAs (shorter kernel tail)
                    ob = outev.tile([128, DIM], BF16, tag="ob", name="ob")
                    drain_at = (1, 3, 5, 7) if fine else (3, 7)
                    span = 2 if fine else 4
                    for nb in range(8):
                        o_ps = psC.tile([128, 512], F32, tag="ops", name="ops")
                        for h2 in range(QH):
                            nc.tensor.matmul(
                                o_ps,
                                ot_hist[qbc][h2][:, qc * 128:(qc + 1) * 128],
                                wo_sb[:, h2 * DIM + nb * 512:
                                      h2 * DIM + (nb + 1) * 512],
                                start=(h2 == 0), stop=(h2 == QH - 1),
                            )
                        osl = ob[:, nb * 512:(nb + 1) * 512]
                        if nb % 2 == 0:
                            nc.scalar.copy(osl, o_ps)
                        else:
                            nc.vector.tensor_copy(osl, o_ps)
                        if nb in drain_at:  # drain as soon as ready
                            rows = slice(qbc * 512 + qc * 128,
                                         qbc * 512 + (qc + 1) * 128)
                            cols = slice((nb - span + 1) * 512,
                                         (nb + 1) * 512)
                            nc.sync.dma_start(out=out[rows, cols],
                                              in_=ob[:, cols])

                # descending qb: the first slot (which has no C work yet to
                # fill the PE during the normalization chains) is the longest,
                # densest stream; C(qb_prev) then interleaves into each later
                # slot, and C(0) drains at the end
                qb_order = [3, 2, 1, 0]
                for slot, qb in enumerate(qb_order):
                    qb_prev = qb_order[slot - 1] if slot >= 1 else None
                    qs = slice(qb * 512, (qb + 1) * 512)
                    n_kb = 4 * qb + 4
                    for h in range(QH):
                        d_ps = psD.tile([128, 512], F32, tag="dps", name="dps")
                        ot_ps = psOT.tile([128, 512], F32, tag="otps",
                                          name="otps")
                        ess = [None] * n_kb

                        def drain(kb):
                            es, qoff, vw = ess[kb]
                            st = (kb == 0)
                            sp = (kb == n_kb - 1)
                            nc.tensor.matmul(
                                d_ps[:, qoff:512], ones_sb, es[:, 0:vw],
                                start=st, stop=sp,
                            )
                            nc.tensor.matmul(
                                ot_ps[:, qoff:512], vN[:, kb, :], es[:, 0:vw],
                                start=st, stop=sp,
                            )

                        for kb in range(n_kb):
                            # diagonal blocks: queries below the staircase see
                            # no valid key -> stream only the live columns
                            j = kb - 4 * qb
                            qoff = 128 * j if j > 0 else 0
                            vw = 512 - qoff
                            s_ps = psS.tile([128, 512], F32, tag="sps",
                                            name="sps")
                            nc.tensor.matmul(
                                s_ps[:, 0:vw],
                                kT[:, kb * 128:(kb + 1) * 128],
                                qT[:, h, qb * 512 + qoff:(qb + 1) * 512],
                                start=True, stop=True,
                            )
                            if j >= 0:  # causal staircase within the block
                                nc.vector.tensor_add(
                                    s_ps[:, 0:vw], s_ps[:, 0:vw],
                                    stair_sb[:, 384:384 + vw],
                                )
                            es = expp.tile([128, 512], BF16, tag="es",
                                           name="es")
                            nc.scalar.activation(
                                es[:, 0:vw], s_ps[:, 0:vw],
                                mybir.ActivationFunctionType.Exp,
                                scale=SCALE,
                            )
                            ess[kb] = (es, qoff, vw)
                            if kb >= LAG:
                                drain(kb - LAG)
                        for kb in range(max(0, n_kb - LAG), n_kb):
                            drain(kb)

                        # normalization: D rows are already broadcast across
                        # all 128 partitions (all-ones lhsT), so 1/D is a
                        # straight elementwise op feeding the O^T scaling
                        rd = rdp.tile([128, 512], F32, tag="rd", name="rd")
                        nc.vector.reciprocal_approx_fast(rd, d_ps)
                        ot = otp.tile([128, 512], BF16, tag=f"ot{h}",
                                      name=f"ot{h}")
                        nc.vector.tensor_mul(ot, ot_ps, rd)
                        ot_hist[qb][h] = ot

                        # keep the PE fed while the chain above retires:
                        # one quarter of the previous slot's out-projection
                        if qb_prev is not None:
                            c_chunk(qb_prev, h)

                for qc in range(4):
                    c_chunk(qb_order[-1], qc, fine=(qc == 3))

    nc.finalize()
    return nc


_NC_CACHE = {}


def _get_nc():
    if "nc" not in _NC_CACHE:
        _NC_CACHE["nc"] = build_nc()
    return _NC_CACHE["nc"]


def _host_prep(x, cos, sin, mask, wq, wk, wv, wo):
    xT = np.ascontiguousarray(x[0].T.astype(np.float32))  # [DIM, SEQ]
    # x chunk (sb, g) holds k-tiles 4g..4g+3, seq cols [512sb, 512sb+512):
    # layout [sb, g, p, i, s'] so each DMA partition line is 4KB contiguous
    x5 = xT.reshape(NCH, 4, 128, NSB, 512)        # [g, i, p, sb, s']
    xPf = np.transpose(x5, (3, 0, 2, 1, 4))       # [sb, g, p, i, s']
    xP = np.ascontiguousarray(
        xPf.astype(BF_NP).reshape(NSB * NCH * 128, 2048))

    cosT = cos[:, 0, :].T.astype(np.float32)
    sinT = sin[:, 0, :].T.astype(np.float32)
    sinTs = np.concatenate([-sinT[:64], sinT[64:]], axis=0)
    csP = np.ascontiguousarray(np.concatenate([cosT, sinTs], axis=1))

    rr = np.arange(128, dtype=np.int64)[:, None]
    cc = np.arange(896, dtype=np.int64)[None, :]
    stair = np.where(rr <= cc - 384, 0.0, NEG).astype(np.float32)
    ones = np.ones((128, 128), dtype=BF_NP)

    def pack_w(w_slice, m):
        # [DIM, m] -> [p, g, i, m] with per-partition contiguous (g, i, m)
        w4 = w_slice.reshape(NCH, 4, 128, m)       # [g, i, p, m]
        wf = np.transpose(w4, (2, 0, 1, 3))        # [p, g, i, m]
        return np.ascontiguousarray(
            wf.astype(BF_NP).reshape(128, NCH * 4 * m))

    in_maps = []
    for i in range(N_CORES):
        wkp = pack_w(wk[:, i * 128:(i + 1) * 128], 128)  # [128, g*512]
        wvp = pack_w(wv[:, i * 128:(i + 1) * 128], 128)
        wkv = np.ascontiguousarray(np.concatenate(
            [wkp.reshape(128, NCH, 512), wvp.reshape(128, NCH, 512)],
            axis=2).reshape(128, NCH * 1024))
        wo_c = wo[i * QS:(i + 1) * QS, :]          # [512, DIM]
        wo4 = wo_c.reshape(QH, 128, DIM)           # [h, p, n]
        woPf = np.ascontiguousarray(
            np.transpose(wo4, (1, 0, 2)).astype(BF_NP).reshape(128, QH * DIM))
        in_maps.append({
            "xP": xP,
            "wqP": pack_w(wq[:, i * QS:(i + 1) * QS], 512),
            "wkvP": wkv,
            "woP": woPf,
            "csP": csP,
            "stairP": stair,
            "onesP": ones,
        })
    return in_maps


def kernel(x, cos, sin, mask, wq, wk, wv, wo, _trace=False, _trace_kwargs=None):
    nc = _get_nc()
    in_maps = _host_prep(x, cos, sin, mask, wq, wk, wv, wo)
    res = run_bass_kernel_spmd(
        nc, in_maps, list(range(N_CORES)), trace=_trace,
        **(_trace_kwargs or {}),
    )
    partials = [np.asarray(res.results[i]["out"], dtype=np.float32)
                for i in range(N_CORES)]
    full = np.sum(np.stack(partials, axis=0), axis=0, dtype=np.float64)
    out = full.astype(np.float32)[None, :, :]
    if _trace:
        return out, res
    return out


# revision 28
# speedup vs baseline: 1.1939x; 1.1939x over previous
"""GQA attention (SEQ=2048, DIM=4096, 32 Q heads / 8 KV heads, head_dim=128),
tensor-parallel over heads across 8 NeuronCores.

Each core owns 4 Q heads + 1 KV head: wq/wk/wv split column-wise, wo split
row-wise; each core produces a partial (2048, 4096) output that the host sums
(the all-reduce of row-parallel wo).

~394us at the full 2.4GHz PE clock (vs the f32r baseline at 629us; PE
roofline for this decomposition is ~388us: 166us QKV + 89us causal-trimmed
attention + 111us out-projection + ~13us startup fill + ~6us tail). Note the
chip drops the PE to 2.0GHz (P0 power state) under sustained load -
back-to-back benchmark runs measure ~470us with an identical instruction
schedule. Changes vs the baseline:
 - all matmul operands in bf16 (same PE rate as f32r, but FWL weight loads,
   half the DMA traffic and half the SBUF footprint; PSUM accum stays f32)
 - host pre-packs every DRAM tensor so each DMA's per-partition line is
   contiguous (>=4KB descriptors), x/out on the sync HWDGE ring, weights and
   tables on the scalar HWDGE ring
 - phase A runs K/V matmuls of each seq block before the Q matmuls and
   double-buffers the K/V PSUM banks, so the next block's PE work never waits
   on the PSUM eviction (RoPE) of the previous one; V^T->V transposes moved
   off the PE onto the DMA xbar (dma_start_transpose)
 - softmax denominator accumulated as broadcast rows via an all-ones lhsT
   (same PE cost, no separate 1->128 broadcast matmul) and inverted with
   reciprocal_approx_fast (~5x faster than reciprocal, which was a 3.3us
   critical-path stall per head)
 - phase C (out-projection) for query block qb is issued interleaved into the
   attention streams of qb+1, so the PE never idles on the normalization
   chain; PSUM evictions alternate between ACT and DVE
"""

import numpy as np
import ml_dtypes

import concourse.bacc as bacc
import concourse.tile as tile
from concourse import mybir
from concourse.bass_utils import run_bass_kernel_spmd

F32 = mybir.dt.float32
BF16 = mybir.dt.bfloat16
BF_NP = ml_dtypes.bfloat16

DIM = 4096
SEQ = 2048
HEAD_DIM = 128
N_CORES = 8
QH = 4              # q heads per core
QS = QH * HEAD_DIM  # 512: wq column slice per core
NKT = DIM // 128    # 32 contraction tiles
NSB = SEQ // 512    # 4 sequence blocks
NCH = 8             # x/weight super-chunks per seq block (4 k-tiles each)
SCALE = 1.0 / float(np.sqrt(HEAD_DIM))
NEG = -1e9
LAG = 5             # D/AV matmuls trail the score stream by LAG blocks


def build_nc():
    nc = bacc.Bacc(trn_type="TRN2")

    xP = nc.declare_dram_parameter("xP", [NSB * NCH * 128, 2048], BF16, isOutput=False)
    # weights packed so one DMA instruction covers multiple super-chunks
    # (the HWDGE ~600ns/instruction issue rate starves the startup otherwise)
    wqP = nc.declare_dram_parameter("wqP", [128, NCH * 2048], BF16, isOutput=False)
    wkvP = nc.declare_dram_parameter("wkvP", [128, NCH * 1024], BF16, isOutput=False)
    woP = nc.declare_dram_parameter("woP", [128, QH * DIM], BF16, isOutput=False)
    csP = nc.declare_dram_parameter("csP", [128, 2 * SEQ], F32, isOutput=False)
    stairP = nc.declare_dram_parameter("stairP", [128, 896], F32, isOutput=False)
    onesP = nc.declare_dram_parameter("onesP", [128, 128], BF16, isOutput=False)
    out = nc.declare_dram_parameter("out", [SEQ, DIM], BF16, isOutput=True)

    xr = xP.rearrange("(c p) s -> p c s", p=128)

    with tile.TileContext(nc) as tc:
        with (
            tc.tile_pool(name="persist", bufs=1) as persist,
            tc.tile_pool(name="resid", bufs=1) as resid,
        ):
            # stair/ones are only needed in phase B; they ride at the tail of
            # the scalar ring so the sync ring starts with the x stream
            stair_sb = persist.tile([128, 896], F32)
            ones_sb = persist.tile([128, 128], BF16)
            wo_sb = persist.tile([128, QH * DIM], BF16)

            # resident activations (all bf16)
            qT = resid.tile([128, QH, SEQ], BF16)      # Q^T per head (d, seq)
            kT = resid.tile([128, SEQ], BF16)          # K^T (d, seq)
            vN = resid.tile([128, SEQ // 128, 128], BF16)  # V natural (keys, d)

            # ---------------- Phase A: projections + RoPE ----------------
            with (
                tc.tile_pool(name="wpool", bufs=1) as wpool,
                tc.tile_pool(name="xpool", bufs=1) as xpool,
                tc.tile_pool(name="cspool", bufs=1) as cspool,
                tc.tile_pool(name="ropetmp", bufs=2) as ropetmp,
                tc.tile_pool(name="vtb", bufs=2) as vtb,
                tc.tile_pool(name="psQ", bufs=1, space="PSUM") as psQ,
                tc.tile_pool(name="psKV", bufs=2, space="PSUM") as psKV,
            ):
                cs_sb = cspool.tile([128, 2 * SEQ], F32)

                wq_sb = wpool.tile([128, NCH * 2048], BF16, name="wq")
                wkv_sb = wpool.tile([128, NCH * 1024], BF16, name="wkv")

                xts = [None] * NCH

                def rope(dst, src_ps, ss):
                    # rotate-half via two partition-offset PSUM->SBUF copies
                    # on ACT; multiplies/add on DVE (sin sign pre-folded)
                    vr = ropetmp.tile([128, 512], F32, tag="vr", name="vr")
                    nc.scalar.copy(vr[0:64, :], src_ps[64:128, :])
                    nc.scalar.copy(vr[64:128, :], src_ps[0:64, :])
                    t = ropetmp.tile([128, 512], F32, tag="t", name="t")
                    u = ropetmp.tile([128, 512], F32, tag="u", name="u")
                    nc.vector.tensor_mul(t, src_ps, cs_sb[:, ss])
                    nc.vector.tensor_mul(
                        u, vr, cs_sb[:, SEQ + ss.start:SEQ + ss.stop])
                    nc.vector.tensor_add(dst, t, u)

                for sb in range(NSB):
                    ss = slice(sb * 512, (sb + 1) * 512)
                    q_ps = [psQ.tile([128, 512], F32, tag=f"qps{h}", name=f"qps{h}")
                            for h in range(QH)]
                    k_ps = psKV.tile([128, 512], F32, tag="kps", name="kps")
                    v_ps = psKV.tile([128, 512], F32, tag="vps", name="vps")

                    # DMAs for this seq block: x chunks on the sync ring;
                    # (first block only) weights on the scalar ring in
                    # consumption order (kv pass g0-3 / q pass g0-3 / ...),
                    # then RoPE tables, then wo
                    for g in range(NCH):
                        xt = xpool.tile([128, 2048], BF16, tag=f"x{g}",
                                        name=f"x{g}")
                        if sb == 0 and g == 0:
                            # prime the pipe: the first matmuls only need the
                            # first k-tile slices; the first two wkv chunks
                            # ride this (otherwise idle) ring in between
                            for i4 in range(4):
                                sl = slice(i4 * 512, (i4 + 1) * 512)
                                nc.sync.dma_start(
                                    out=xt[:, sl], in_=xr[:, 0, sl])
                                if i4 < 2:
                                    kv1 = slice(i4 * 1024, (i4 + 1) * 1024)
                                    nc.sync.dma_start(out=wkv_sb[:, kv1],
                                                      in_=wkvP[:, kv1])
                        else:
                            nc.sync.dma_start(out=xt,
                                              in_=xr[:, sb * NCH + g, :])
                        xts[g] = xt
                    if sb == 0:
                        # weight slices in consumption order; first ones small
                        # so the very first matmuls start ASAP, later ones
                        # coalesced (HWDGE issues ~600ns/instruction)
                        def wkv_dma(glo, ghi):
                            sl = slice(glo * 1024, ghi * 1024)
                            nc.scalar.dma_start(out=wkv_sb[:, sl],
                                                in_=wkvP[:, sl])

                        def wq_dma(glo, ghi):
                            sl = slice(glo * 2048, ghi * 2048)
                            nc.scalar.dma_start(out=wq_sb[:, sl],
                                                in_=wqP[:, sl])

                        wq_dma(0, 1)
                        wkv_dma(2, 4)
                        wq_dma(1, 2)
                        wq_dma(2, 4)
                        wkv_dma(4, 6)
                        wkv_dma(6, 8)
                        wq_dma(4, 6)
                        wq_dma(6, 8)
                        nc.scalar.dma_start(out=cs_sb, in_=csP[:, :])
                        nc.scalar.dma_start(out=stair_sb, in_=stairP[:, :])
                        nc.scalar.dma_start(out=ones_sb, in_=onesP[:, :])
                        for c in range(2):
                            cs = slice(c * QH * DIM // 2, (c + 1) * QH * DIM // 2)
                            nc.scalar.dma_start(out=wo_sb[:, cs], in_=woP[:, cs])

                    def kv_pass(gs):
                        for g in gs:
                            for i in range(4):
                                kt_i = g * 4 + i
                                st = (kt_i == 0)
                                sp = (kt_i == NKT - 1)
                                xsl = xts[g][:, i * 512:(i + 1) * 512]
                                w0 = g * 1024 + i * 128
                                nc.tensor.matmul(
                                    k_ps, wkv_sb[:, w0:w0 + 128],
                                    xsl, start=st, stop=sp,
                                )
                                nc.tensor.matmul(
                                    v_ps, wkv_sb[:, w0 + 512:w0 + 640],
                                    xsl, start=st, stop=sp,
                                )

                    def q_pass(gs):
                        for g in gs:
                            for i in range(4):
                                kt_i = g * 4 + i
                                st = (kt_i == 0)
                                sp = (kt_i == NKT - 1)
                                xsl = xts[g][:, i * 512:(i + 1) * 512]
                                for h in range(QH):
                                    w0 = g * 2048 + i * 512 + h * 128
                                    nc.tensor.matmul(
                                        q_ps[h], wq_sb[:, w0:w0 + 128],
                                        xsl, start=st, stop=sp,
                                    )

                    # K/V of the next block never waits on Q evictions, and
                    # Q evictions overlap the second-half Q matmuls
                    kv_pass(range(0, 4))
                    q_pass(range(0, 4))
                    kv_pass(range(4, 8))

                    # K/V eviction (K rope + V transpose via DMA xbar),
                    # overlapped with the second-half Q matmuls below
                    rope(kT[:, ss], k_ps, ss)
                    vt_sb = vtb.tile([128, 512], BF16, tag="vt", name="vt")
                    nc.scalar.copy(vt_sb, v_ps)
                    for j in range(4):
                        nc.sync.dma_start_transpose(
                            out=vN[:, sb * 4 + j, :],
                            in_=vt_sb[:, j * 128:(j + 1) * 128],
                        )

                    # second-half Q pass head-major: head h's RoPE (which
                    # frees its PSUM bank and finishes its qT slice) runs
                    # under head h+1's matmul stream, so neither the next
                    # seq block nor phase B ever waits on an eviction tail
                    for h in range(QH):
                        for g in range(4, 8):
                            for i in range(4):
                                kt_i = g * 4 + i
                                w0 = g * 2048 + i * 512 + h * 128
                                nc.tensor.matmul(
                                    q_ps[h], wq_sb[:, w0:w0 + 128],
                                    xts[g][:, i * 512:(i + 1) * 512],
                                    start=False, stop=(kt_i == NKT - 1),
                                )
                        rope(qT[:, h, ss], q_ps[h], ss)

            # ---------------- Phase B/C: attention + out projection ----------------
            with (
                tc.tile_pool(name="expp", bufs=10) as expp,
                tc.tile_pool(name="otp", bufs=2) as otp,
                tc.tile_pool(name="rdp", bufs=2) as rdp,
                tc.tile_pool(name="outev", bufs=3) as outev,
                # pool-open order controls bank placement: psD/psOT/psC take
                # the q-accumulator banks (whose RoPE evictions drain last but
                # are only needed ~LAG blocks into phase B), psS gets the k/v
                # banks which phase A freed long before its end
                tc.tile_pool(name="psD", bufs=1, space="PSUM") as psD,
                tc.tile_pool(name="psOT", bufs=1, space="PSUM") as psOT,
                tc.tile_pool(name="psC", bufs=2, space="PSUM") as psC,
                tc.tile_pool(name="psS", bufs=4, space="PSUM") as psS,
            ):
                ot_hist = [[None] * QH for _ in range(NSB)]

                def c_chunk(qbc, qc, fine=False):
                    # out rows [qbc*512 + qc*128, +128) x all 4096 cols;
                    # fine=True drains quarter-DMAs (shorter kernel tail)
                    ob = outev.tile([128, DIM], BF16, tag="ob", name="ob")
                    drain_at = (1, 3, 5, 7) if fine else (3, 7)
                    span = 2 if fine else 4
                    for nb in range(8):
                        o_ps = psC.tile([128, 512], F32, tag="ops", name="ops")
                        for h2 in range(QH):
                            nc.tensor.matmul(
                                o_ps,
                                ot_hist[qbc][h2][:, qc * 128:(qc + 1) * 128],
                                wo_sb[:, h2 * DIM + nb * 512:
                                      h2 * DIM + (nb + 1) * 512],
                                start=(h2 == 0), stop=(h2 == QH - 1),
                            )
                        osl = ob[:, nb * 512:(nb + 1) * 512]
                        if nb % 2 == 0:
                            nc.scalar.copy(osl, o_ps)
                        else:
                            nc.vector.tensor_copy(osl, o_ps)
                        if nb in drain_at:  # drain as soon as ready
                            rows = slice(qbc * 512 + qc * 128,
                                         qbc * 512 + (qc + 1) * 128)
                            cols = slice((nb - span + 1) * 512,
                                         (nb + 1) * 512)
                            nc.sync.dma_start(out=out[rows, cols],
                                              in_=ob[:, cols])

                # descending qb: the first slot (which has no C work yet to
                # fill the PE during the normalization chains) is the longest,
                # densest stream; C(qb_prev) then interleaves into each later
                # slot, and C(0) drains at the end
                qb_order = [3, 2, 1, 0]
                for slot, qb in enumerate(qb_order):
                    qb_prev = qb_order[slot - 1] if slot >= 1 else None
                    qs = slice(qb * 512, (qb + 1) * 512)
                    n_kb = 4 * qb + 4
                    for h in range(QH):
                        d_ps = psD.tile([128, 512], F32, tag="dps", name="dps")
                        ot_ps = psOT.tile([128, 512], F32, tag="otps",
                                          name="otps")
                        ess = [None] * n_kb

                        def drain(kb):
                            es, qoff, vw = ess[kb]
                            st = (kb == 0)
                            sp = (kb == n_kb - 1)
                            nc.tensor.matmul(
                                d_ps[:, qoff:512], ones_sb, es[:, 0:vw],
                                start=st, stop=sp,
                            )
                            nc.tensor.matmul(
                                ot_ps[:, qoff:512], vN[:, kb, :], es[:, 0:vw],
                                start=st, stop=sp,
                            )

                        for kb in range(n_kb):
                            # diagonal blocks: queries below the staircase see
                            # no valid key -> stream only the live columns
                            j = kb - 4 * qb
                            qoff = 128 * j if j > 0 else 0
                            vw = 512 - qoff
                            s_ps = psS.tile([128, 512], F32, tag="sps",
                                            name="sps")
                            nc.tensor.matmul(
                                s_ps[:, 0:vw],
                                kT[:, kb * 128:(kb + 1) * 128],
                                qT[:, h, qb * 512 + qoff:(qb + 1) * 512],
                                start=True, stop=True,
                            )
                            if j >= 0:  # causal staircase within the block
                                nc.vector.tensor_add(
                                    s_ps[:, 0:vw], s_ps[:, 0:vw],
                                    stair_sb[:, 384:384 + vw],
                                )
                            es = expp.tile([128, 512], BF16, tag="es",
                                           name="es")
                            nc.scalar.activation(
                                es[:, 0:vw], s_ps[:, 0:vw],
                                mybir.ActivationFunctionType.Exp,
                                scale=SCALE,
                            )
                            ess[kb] = (es, qoff, vw)
                            if kb >= LAG:
                                drain(kb - LAG)
                        for kb in range(max(0, n_kb - LAG), n_kb):
                            drain(kb)

                        # normalization: D rows are already broadcast across
                        # all 128 partitions (all-ones lhsT), so 1/D is a
                        # straight elementwise op feeding the O^T scaling
                        rd = rdp.tile([128, 512], F32, tag="rd", name="rd")
                        nc.vector.reciprocal_approx_fast(rd, d_ps)
                        ot = otp.tile([128, 512], BF16, tag=f"ot{h}",
                                      name=f"ot{h}")
                        nc.vector.tensor_mul(ot, ot_ps, rd)
                        ot_hist[qb][h] = ot

                        # keep the PE fed while the chain above retires:
                        # one quarter of the previous slot's out-projection
                        if qb_prev is not None:
                            c_chunk(qb_prev, h)

                for qc in range(4):
                    c_chunk(qb_order[-1], qc, fine=(qc == 3))

    nc.finalize()
    return nc


_NC_CACHE = {}


def _get_nc():
    if "nc" not in _NC_CACHE:
        _NC_CACHE["nc"] = build_nc()
    return _NC_CACHE["nc"]


def _host_prep(x, cos, sin, mask, wq, wk, wv, wo):
    xT = np.ascontiguousarray(x[0].T.astype(np.float32))  # [DIM, SEQ]
    # x chunk (sb, g) holds k-tiles 4g..4g+3, seq cols [512sb, 512sb+512):
    # layout [sb, g, p, i, s'] so each DMA partition line is 4KB contiguous
    x5 = xT.reshape(NCH, 4, 128, NSB, 512)        # [g, i, p, sb, s']
    xPf = np.transpose(x5, (3, 0, 2, 1, 4))       # [sb, g, p, i, s']
    xP = np.ascontiguousarray(
        xPf.astype(BF_NP).reshape(NSB * NCH * 128, 2048))

    cosT = cos[:, 0, :].T.astype(np.float32)
    sinT = sin[:, 0, :].T.astype(np.float32)
    sinTs = np.concatenate([-sinT[:64], sinT[64:]], axis=0)
    csP = np.ascontiguousarray(np.concatenate([cosT, sinTs], axis=1))

    rr = np.arange(128, dtype=np.int64)[:, None]
    cc = np.arange(896, dtype=np.int64)[None, :]
    stair = np.where(rr <= cc - 384, 0.0, NEG).astype(np.float32)
    ones = np.ones((128, 128), dtype=BF_NP)

    def pack_w(w_slice, m):
        # [DIM, m] -> [p, g, i, m] with per-partition contiguous (g, i, m)
        w4 = w_slice.reshape(NCH, 4, 128, m)       # [g, i, p, m]
        wf = np.transpose(w4, (2, 0, 1, 3))        # [p, g, i, m]
        return np.ascontiguousarray(
            wf.astype(BF_NP).reshape(128, NCH * 4 * m))

    in_maps = []
    for i in range(N_CORES):
        wkp = pack_w(wk[:, i * 128:(i + 1) * 128], 128)  # [128, g*512]
        wvp = pack_w(wv[:, i * 128:(i + 1) * 128], 128)
        wkv = np.ascontiguousarray(np.concatenate(
            [wkp.reshape(128, NCH, 512), wvp.reshape(128, NCH, 512)],
            axis=2).reshape(128, NCH * 1024))
        wo_c = wo[i * QS:(i + 1) * QS, :]          # [512, DIM]
        wo4 = wo_c.reshape(QH, 128, DIM)           # [h, p, n]
        woPf = np.ascontiguousarray(
            np.transpose(wo4, (1, 0, 2)).astype(BF_NP).reshape(128, QH * DIM))
        in_maps.append({
            "xP": xP,
            "wqP": pack_w(wq[:, i * QS:(i + 1) * QS], 512),
            "wkvP": wkv,
            "woP": woPf,
            "csP": csP,
            "stairP": stair,
            "onesP": ones,
        })
    return in_maps


def kernel(x, cos, sin, mask, wq, wk, wv, wo, _trace=False, _trace_kwargs=None):
    nc = _get_nc()
    in_maps = _host_prep(x, cos, sin, mask, wq, wk, wv, wo)
    res = run_bass_kernel_spmd(
        nc, in_maps, list(range(N_CORES)), trace=_trace,
        **(_trace_kwargs or {}),
    )
    partials = [np.asarray(res.results[i]["out"], dtype=np.float32)
                for i in range(N_CORES)]
    full = np.sum(np.stack(partials, axis=0), axis=0, dtype=np.float64)
    out = full.astype(np.float32)[None, :, :]
    if _trace:
        return out, res
    return out
